# revision 2
# baseline (speedup 1.0000x reference)
"""Trainium2 Bass kernel for nn_AdaptiveComputationGraph (moe_routing).

Strategy
--------
The router (tiny scalar MLP on normalized per-sample uncertainty) is computed
on host in numpy (sub-millisecond, ~10 MFLOP vs 275 GFLOP of layer compute).
The heavy compute -- the depth-4 chain of [B,D]@[D,D] GEMMs + GELU -- runs
dense on all 8 NeuronCores, data-parallel over the batch (1024 rows/core),
replicated weights, zero cross-core communication.

The device kernel keeps activations FEATURE-MAJOR (transposed, [D, rows]) so
every layer is out^T = gelu(W^T @ in^T + b) with the contraction dim on SBUF
partitions for both operands -- no transposes anywhere in the chain.  Compute
dtype is bf16 (4x TensorE rate vs fp32), fp32 PSUM accumulation, gelu+bias
fused into the PSUM->SBUF eviction on ScalarE.

The kernel outputs the level-1, level-2 and level-4 activations (h1, h2, h4)
for every row; the host selects per row according to the routing level and
re-transposes.  This is correct for any routing distribution and
FLOP-minimal for the actual one (routing sends 8191/8192 samples to the
deepest level, so per-sample early exit would save <0.01% FLOPs).
"""

import sys

sys.path.insert(0, "/opt/trn_rl_repo")

import numpy as np
import ml_dtypes


def _ensure_ntff_hook():
    """concourse's axon trace path reads antenv.axon_hooks, which this image's
    antenv package lacks.  Install an equivalent shim backed by the ctypes
    NTFF driver from trn_agent_boot so trace=True / BASS_TRACE=1 can profile.
    No-op if the real module (or a previous shim) is importable."""
    try:
        import antenv.axon_hooks  # noqa: F401
        return
    except ImportError:
        pass
    try:
        import types
        import antenv
        from trn_agent_boot.trn_boot import _ntff_profile_via_ctypes

        hook = _ntff_profile_via_ctypes("/opt/axon/libaxon_pjrt.so")
        mod = types.ModuleType("antenv.axon_hooks")
        mod._hook = hook
        mod.get_axon_ntff_profile_hook = lambda: mod._hook
        mod.set_axon_ntff_profile_hook = lambda h: setattr(mod, "_hook", h)
        sys.modules["antenv.axon_hooks"] = mod
        antenv.axon_hooks = mod
    except Exception:
        pass


_ensure_ntff_hook()

N_CORES = 8
B, D, L = 8192, 2048, 4
R = B // N_CORES  # rows per core
P = 128  # SBUF partitions
KT = D // P  # contraction tiles per layer
MT = D // P  # output-feature blocks per layer
RC = 512  # rows per matmul (PSUM bank = 512 fp32)
NRC = R // RC

_COMPILED = None
LAST_RESULTS = None  # BassKernelResults of the most recent run (for profiling)


def _build():
    from concourse import bacc, mybir, tile

    nc = bacc.Bacc("TRN2", target_bir_lowering=False, debug=False,
                   num_devices=N_CORES)
    bf16 = mybir.dt.bfloat16
    f32 = mybir.dt.float32
    gelu = mybir.ActivationFunctionType.Gelu_apprx_tanh

    xt_ext = nc.declare_dram_parameter("xt", [D, R], bf16, isOutput=False)
    w_ext = nc.declare_dram_parameter("w", [L, D, D], bf16, isOutput=False)
    br_ext = nc.declare_dram_parameter("br", [L, P, MT], f32, isOutput=False)
    h1_ext = nc.declare_dram_parameter("h1", [D, R], bf16, isOutput=True)
    h2_ext = nc.declare_dram_parameter("h2", [D, R], bf16, isOutput=True)
    h4_ext = nc.declare_dram_parameter("h4", [D, R], bf16, isOutput=True)
    outs = {0: h1_ext, 1: h2_ext, 3: h4_ext}

    with tile.TileContext(nc) as tc:
        with (
            tc.tile_pool(name="acts", bufs=48) as act_pool,
            tc.tile_pool(name="wpool", bufs=20) as w_pool,
            tc.tile_pool(name="bias", bufs=4) as b_pool,
            tc.tile_pool(name="psum", bufs=8, space="PSUM") as psum_pool,
        ):
            bias_tiles = []
            for l in range(L):
                bt = b_pool.tile([P, MT], f32, name=f"bias{l}", tag="bias")
                nc.sync.dma_start(out=bt[:], in_=br_ext[l])
                bias_tiles.append(bt)

            cur = []
            for k in range(KT):
                t = act_pool.tile([P, R], bf16, name=f"x{k}", tag="act")
                nc.sync.dma_start(out=t[:], in_=xt_ext[k * P:(k + 1) * P, :])
                cur.append(t)

            for l in range(L):
                wts = []
                for k in range(KT):
                    wt = w_pool.tile([P, D], bf16, name=f"w{l}_{k}", tag="w")
                    nc.sync.dma_start(out=wt[:], in_=w_ext[l, k * P:(k + 1) * P, :])
                    wts.append(wt)
                nxt = []
                for m in range(MT):
                    ot = act_pool.tile([P, R], bf16, name=f"h{l}_{m}", tag="act")
                    psums = [
                        psum_pool.tile([P, RC], f32, name=f"ps{l}_{m}_{r}", tag="ps")
                        for r in range(NRC)
                    ]
                    for k in range(KT):
                        wap = wts[k][:, m * P:(m + 1) * P]
                        for r in range(NRC):
                            nc.tensor.matmul(
                                psums[r][:], wap, cur[k][:, r * RC:(r + 1) * RC],
                                start=(k == 0), stop=(k == KT - 1),
                            )
                    for r in range(NRC):
                        nc.scalar.activation(
                            ot[:, r * RC:(r + 1) * RC], psums[r][:], gelu,
                            bias=bias_tiles[l][:, m:m + 1],
                        )
                    if l in outs:
                        nc.sync.dma_start(
                            out=outs[l][m * P:(m + 1) * P, :], in_=ot[:]
                        )
                    nxt.append(ot)
                cur = nxt

    nc.compile()
    return nc


def _get_compiled():
    global _COMPILED
    if _COMPILED is None:
        _COMPILED = _build()
    return _COMPILED


def _route_np(unc, rw1, rb1, rw2, rb2, rw3, rb3):
    """Replicates reference._route in float32 numpy (argmax of softmax ==
    argmax of logits)."""
    unc = unc.astype(np.float32)
    u = (unc - unc.min()) / (unc.max() - unc.min() + np.float32(1e-8))
    h = np.maximum(u[:, None] * rw1[0][None, :] + rb1, np.float32(0))
    h = np.maximum(h @ rw2 + rb2, np.float32(0))
    logits = h @ rw3 + rb3
    return np.argmax(logits, axis=-1)


def kernel(x, current_uncertainty, Ws, bs, rw1, rb1, rw2, rb2, rw3, rb3):
    global LAST_RESULTS
    from concourse.bass_utils import run_bass_kernel_spmd

    x = np.asarray(x, dtype=np.float32)
    Ws = np.asarray(Ws, dtype=np.float32)
    bs = np.asarray(bs, dtype=np.float32)

    routing = _route_np(
        np.asarray(current_uncertainty, dtype=np.float32),
        np.asarray(rw1, dtype=np.float32), np.asarray(rb1, dtype=np.float32),
        np.asarray(rw2, dtype=np.float32), np.asarray(rb2, dtype=np.float32),
        np.asarray(rw3, dtype=np.float32), np.asarray(rb3, dtype=np.float32),
    )

    bf = ml_dtypes.bfloat16
    xt = np.ascontiguousarray(x.T).astype(bf)  # [D, B]
    w_bf = np.ascontiguousarray(Ws).astype(bf)  # [L, D, D]
    # bias rearranged so partition p of feature-block m holds bs[l, m*128+p]
    br = np.ascontiguousarray(
        bs.reshape(L, MT, P).transpose(0, 2, 1)
    ).astype(np.float32)  # [L, P, MT]

    in_maps = [
        {
            "xt": np.ascontiguousarray(xt[:, c * R:(c + 1) * R]),
            "w": w_bf,
            "br": br,
        }
        for c in range(N_CORES)
    ]

    nc = _get_compiled()
    res = run_bass_kernel_spmd(nc, in_maps, list(range(N_CORES)))
    LAST_RESULTS = res

    def gather(name):
        full = np.concatenate(
            [np.asarray(res.results[c][name]) for c in range(N_CORES)], axis=1
        )  # [D, B]
        return np.ascontiguousarray(full.T).astype(np.float32)  # [B, D]

    h1 = gather("h1")
    h2 = gather("h2")
    h4 = gather("h4")

    final = h4
    lvl0 = routing == 0
    lvl1 = routing == 1
    if lvl0.any():
        final[lvl0] = h1[lvl0]
    if lvl1.any():
        final[lvl1] = h2[lvl1]

    mask = routing.astype(np.float32)
    return final, mask


# revision 4
# speedup vs baseline: 1.0243x; 1.0243x over previous
"""Trainium2 Bass kernel for nn_AdaptiveComputationGraph (moe_routing).

Strategy
--------
The router (tiny scalar MLP on normalized per-sample uncertainty) is computed
on host in numpy (sub-millisecond, ~10 MFLOP vs 275 GFLOP of layer compute).
The heavy compute -- the depth-4 chain of [B,D]@[D,D] GEMMs + GELU -- runs
dense on all 8 NeuronCores, data-parallel over the batch (1024 rows/core),
replicated weights, zero cross-core communication.

The device kernel keeps activations FEATURE-MAJOR (transposed, [D, rows]) so
every layer is out^T = gelu(W^T @ in^T + b) with the contraction dim on SBUF
partitions for both operands -- no transposes anywhere in the chain.  Compute
dtype is bf16 (4x TensorE rate vs fp32), fp32 PSUM accumulation, gelu+bias
fused into the PSUM->SBUF eviction on ScalarE.

The kernel outputs the level-1, level-2 and level-4 activations (h1, h2, h4)
for every row; the host selects per row according to the routing level and
re-transposes.  This is correct for any routing distribution and
FLOP-minimal for the actual one (routing sends 8191/8192 samples to the
deepest level, so per-sample early exit would save <0.01% FLOPs).
"""

import sys

sys.path.insert(0, "/opt/trn_rl_repo")

import numpy as np
import ml_dtypes


def _ensure_ntff_hook():
    """concourse's axon trace path reads antenv.axon_hooks, which this image's
    antenv package lacks.  Install an equivalent shim backed by the ctypes
    NTFF driver from trn_agent_boot so trace=True / BASS_TRACE=1 can profile.
    No-op if the real module (or a previous shim) is importable."""
    try:
        import antenv.axon_hooks  # noqa: F401
        return
    except ImportError:
        pass
    try:
        import types
        import antenv
        from trn_agent_boot.trn_boot import _ntff_profile_via_ctypes

        hook = _ntff_profile_via_ctypes("/opt/axon/libaxon_pjrt.so")
        mod = types.ModuleType("antenv.axon_hooks")
        mod._hook = hook
        mod.get_axon_ntff_profile_hook = lambda: mod._hook
        mod.set_axon_ntff_profile_hook = lambda h: setattr(mod, "_hook", h)
        sys.modules["antenv.axon_hooks"] = mod
        antenv.axon_hooks = mod
    except Exception:
        pass


_ensure_ntff_hook()

N_CORES = 8
B, D, L = 8192, 2048, 4
R = B // N_CORES  # rows per core
P = 128  # SBUF partitions
KT = D // P  # contraction tiles per layer
MT = D // P  # output-feature blocks per layer
RC = 512  # rows per matmul (PSUM bank = 512 fp32)
NRC = R // RC

_COMPILED = None
LAST_RESULTS = None  # BassKernelResults of the most recent run (for profiling)


def _build():
    from concourse import bacc, mybir, tile

    nc = bacc.Bacc("TRN2", target_bir_lowering=False, debug=False,
                   num_devices=N_CORES)
    bf16 = mybir.dt.bfloat16
    f32 = mybir.dt.float32
    gelu = mybir.ActivationFunctionType.Gelu_apprx_tanh

    xt_ext = nc.declare_dram_parameter("xt", [D, R], bf16, isOutput=False)
    w_ext = nc.declare_dram_parameter("w", [L, D, D], bf16, isOutput=False)
    br_ext = nc.declare_dram_parameter("br", [L, P, MT], f32, isOutput=False)
    h1_ext = nc.declare_dram_parameter("h1", [D, R], bf16, isOutput=True)
    h2_ext = nc.declare_dram_parameter("h2", [D, R], bf16, isOutput=True)
    h4_ext = nc.declare_dram_parameter("h4", [D, R], bf16, isOutput=True)
    outs = {0: h1_ext, 1: h2_ext, 3: h4_ext}

    with tile.TileContext(nc) as tc:
        with (
            tc.tile_pool(name="acts", bufs=48) as act_pool,
            tc.tile_pool(name="wpool", bufs=40) as w_pool,
            tc.tile_pool(name="bias", bufs=4) as b_pool,
            tc.tile_pool(name="psum", bufs=8, space="PSUM") as psum_pool,
        ):
            bias_tiles = []
            for l in range(L):
                bt = b_pool.tile([P, MT], f32, name=f"bias{l}", tag="bias")
                nc.sync.dma_start(out=bt[:], in_=br_ext[l])
                bias_tiles.append(bt)

            # Weight DMAs are split into column halves [128, 1024] so a
            # single tile's load parallelizes across DMA queues and the
            # m<8 blocks only wait on half 0 -- cuts the startup stall
            # before the first PSUM group can complete.
            HW = D // 2  # columns per weight half

            cur = []
            for k in range(KT):
                t = act_pool.tile([P, R], bf16, name=f"x{k}", tag="act")
                nc.sync.dma_start(out=t[:], in_=xt_ext[k * P:(k + 1) * P, :])
                w0 = w_pool.tile([P, HW], bf16, name=f"w0_{k}_h0", tag="w")
                nc.sync.dma_start(out=w0[:], in_=w_ext[0, k * P:(k + 1) * P, 0:HW])
                if k == 0:
                    wts = [[None, None] for _ in range(KT)]
                wts[k][0] = w0
                cur.append(t)
            for k in range(KT):
                w1 = w_pool.tile([P, HW], bf16, name=f"w0_{k}_h1", tag="w")
                nc.sync.dma_start(out=w1[:], in_=w_ext[0, k * P:(k + 1) * P, HW:D])
                wts[k][1] = w1

            for l in range(L):
                if l > 0:
                    wts = []
                    for k in range(KT):
                        halves = []
                        for h in range(2):
                            wt = w_pool.tile([P, HW], bf16,
                                             name=f"w{l}_{k}_h{h}", tag="w")
                            nc.sync.dma_start(
                                out=wt[:],
                                in_=w_ext[l, k * P:(k + 1) * P, h * HW:(h + 1) * HW],
                            )
                            halves.append(wt)
                        wts.append(halves)
                nxt = []
                for m in range(MT):
                    ot = act_pool.tile([P, R], bf16, name=f"h{l}_{m}", tag="act")
                    psums = [
                        psum_pool.tile([P, RC], f32, name=f"ps{l}_{m}_{r}", tag="ps")
                        for r in range(NRC)
                    ]
                    h, ml = divmod(m, MT // 2)
                    for k in range(KT):
                        wap = wts[k][h][:, ml * P:(ml + 1) * P]
                        for r in range(NRC):
                            nc.tensor.matmul(
                                psums[r][:], wap, cur[k][:, r * RC:(r + 1) * RC],
                                start=(k == 0), stop=(k == KT - 1),
                            )
                    for r in range(NRC):
                        nc.scalar.activation(
                            ot[:, r * RC:(r + 1) * RC], psums[r][:], gelu,
                            bias=bias_tiles[l][:, m:m + 1],
                        )
                    if l in outs:
                        nc.sync.dma_start(
                            out=outs[l][m * P:(m + 1) * P, :], in_=ot[:]
                        )
                    nxt.append(ot)
                cur = nxt

    nc.compile()
    return nc


def _get_compiled():
    global _COMPILED
    if _COMPILED is None:
        _COMPILED = _build()
    return _COMPILED


def _route_np(unc, rw1, rb1, rw2, rb2, rw3, rb3):
    """Replicates reference._route in float32 numpy (argmax of softmax ==
    argmax of logits)."""
    unc = unc.astype(np.float32)
    u = (unc - unc.min()) / (unc.max() - unc.min() + np.float32(1e-8))
    h = np.maximum(u[:, None] * rw1[0][None, :] + rb1, np.float32(0))
    h = np.maximum(h @ rw2 + rb2, np.float32(0))
    logits = h @ rw3 + rb3
    return np.argmax(logits, axis=-1)


def kernel(x, current_uncertainty, Ws, bs, rw1, rb1, rw2, rb2, rw3, rb3):
    global LAST_RESULTS
    from concourse.bass_utils import run_bass_kernel_spmd

    x = np.asarray(x, dtype=np.float32)
    Ws = np.asarray(Ws, dtype=np.float32)
    bs = np.asarray(bs, dtype=np.float32)

    routing = _route_np(
        np.asarray(current_uncertainty, dtype=np.float32),
        np.asarray(rw1, dtype=np.float32), np.asarray(rb1, dtype=np.float32),
        np.asarray(rw2, dtype=np.float32), np.asarray(rb2, dtype=np.float32),
        np.asarray(rw3, dtype=np.float32), np.asarray(rb3, dtype=np.float32),
    )

    bf = ml_dtypes.bfloat16
    xt = np.ascontiguousarray(x.T).astype(bf)  # [D, B]
    w_bf = np.ascontiguousarray(Ws).astype(bf)  # [L, D, D]
    # bias rearranged so partition p of feature-block m holds bs[l, m*128+p]
    br = np.ascontiguousarray(
        bs.reshape(L, MT, P).transpose(0, 2, 1)
    ).astype(np.float32)  # [L, P, MT]

    in_maps = [
        {
            "xt": np.ascontiguousarray(xt[:, c * R:(c + 1) * R]),
            "w": w_bf,
            "br": br,
        }
        for c in range(N_CORES)
    ]

    nc = _get_compiled()
    res = run_bass_kernel_spmd(nc, in_maps, list(range(N_CORES)))
    LAST_RESULTS = res

    def gather(name):
        full = np.concatenate(
            [np.asarray(res.results[c][name]) for c in range(N_CORES)], axis=1
        )  # [D, B]
        return np.ascontiguousarray(full.T).astype(np.float32)  # [B, D]

    h1 = gather("h1")
    h2 = gather("h2")
    h4 = gather("h4")

    final = h4
    lvl0 = routing == 0
    lvl1 = routing == 1
    if lvl0.any():
        final[lvl0] = h1[lvl0]
    if lvl1.any():
        final[lvl1] = h2[lvl1]

    mask = routing.astype(np.float32)
    return final, mask
